# revision 21
# baseline (speedup 1.0000x reference)
"""Sparse masked attention layer for Trainium2, sharded over 8 NeuronCores.

Strategy
--------
The reference masks attention columns (keys) not in ``mask_ind`` with -inf
before softmax and zeroes rows (queries) not in ``mask_ind`` after softmax.
Both facts mean only the ~63% of token positions present in ``mask_ind``
participate at all: rows absent from the set produce exactly ``bproj`` in
the output.  So the host compacts each batch down to its kept token set,
the device runs *dense* attention on the compacted tokens (exactly equal
to the reference's masked softmax), and the host scatters results back,
filling non-kept rows with ``bproj``.

Sharding: core = (batch b, head-group g) -> 4 batches x 2 groups of 8
heads.  Each core computes q/k/v projections for its 8 heads from the
(replicated per-batch) compacted activations, attention per head, and its
partial contribution to the output projection (rows g*512:(g+1)*512 of
Wproj).  The two partials of a batch are summed on the host (D-sharded
matmul reduce) and bproj is added there.

All matmuls run in bfloat16 (fp32 PSUM accumulate); fp32(r) matmuls
measured 2.5-4.3 cycles/row on hardware while bf16 streams at ~1 row per
cycle.  The softmax denominator stays exact w.r.t. pT rounding: the
keep-column trick sums the *same* bf16 pT values the AV matmul consumes.

The key/value side is padded to a multiple of 128 (Cp, PE/PSUM chunking);
the query side only to a multiple of 32 (Cq) - queries never form a
contraction dimension, and every q-sized loop (exp elements, S/AV moving
rows, output projection) scales with it, so the tighter pad saves ~7%.

Schedule (what the traces drove):
- Attention runs S->exp->AV software-pipelined with lookahead 2; the
  measured steady state is ACT- and PE-balanced (~1us per 128-key chunk
  of a head pair), so the projections stay as separate phases - filler
  experiments starved the ACT and lost time.
- At head-pair boundaries the first two S matmuls of the next block are
  interleaved between the AV tail pairs of the previous one, so the exp
  pipeline never drains.
- Softmax normalization runs entirely off-PE: the AV stationary puts the
  keep column at v-column 0 (denominator lands on PSUM partition 0,
  where the DVE fast-reciprocal can read PSUM directly - it malfunctions
  on other partitions) and the v features at columns 64..127 (64-wide
  partition accesses must start at 0 or 64).  DVE reciprocal -> GpSimd
  partition_broadcast -> DVE multiply; no PE instruction in the chain.
- One ACT exp instruction covers a head pair ([128, 2, qsz] strided over
  two PSUM banks), halving ACT instruction overhead; the Exp table is
  preloaded during the projection phase.

Device layouts (per core):
  xT   [D, Cp]   compacted activations, transposed (host-side transpose)
  qkT  [128, 8, Cp] sbuf bf16: chunks 0-3 = q features (first Cq columns
       used), 4-7 = k features
  v    [128, NC, 8, 128] sbuf bf16: per c-chunk, per head: keep column
       at 0, zero pad at 1..63, 64 v-features at 64..127.
  attnT [64, 8, Cq] normalized attention output, transposed - exactly the
       lhsT layout the output projection needs.
"""

import numpy as np
import ml_dtypes

BF16 = ml_dtypes.bfloat16

B, C, D, H = 4, 2048, 1024, 16
HD = D // H          # 64
HPC = H // 2         # 8 heads per core
FQ = HPC * HD        # 512 per-core q/k/v feature count
VW = 128             # v stationary width: keep @0, pad 1-63, v @64-127
N_CORES = 8

_NC_CACHE = {}


def _chunks(total, step):
    return [(i, min(step, total - i)) for i in range(0, total, step)]


def _build_nc(Cp, Cq, with_bias):
    import concourse.mybir as mybir
    import concourse.tile as tile
    from concourse import bacc

    f32 = mybir.dt.float32
    bf16 = mybir.dt.bfloat16
    Exp = mybir.ActivationFunctionType.Exp

    NC = Cp // 128       # kept-token chunks of 128 (key/value side)
    KD = D // 128        # 8 contraction chunks for the projections
    kgroups = _chunks(Cp, 512)
    qgroups = _chunks(Cq, 512)
    G = len(qgroups)
    assert NC >= 3

    nc = bacc.Bacc()
    xT = nc.dram_tensor("xT", [D, Cp], bf16, kind="ExternalInput")
    wqk = nc.dram_tensor("wqk", [D, 2 * FQ], bf16, kind="ExternalInput")
    wv = nc.dram_tensor("wv", [D, FQ], bf16, kind="ExternalInput")
    wp = nc.dram_tensor("wp", [FQ, D], bf16, kind="ExternalInput")
    keep = nc.dram_tensor("keep", [128, NC], f32, kind="ExternalInput")
    keepb = nc.dram_tensor("keepb", [128, NC], bf16, kind="ExternalInput")
    if with_bias:
        bqk = nc.dram_tensor("bqk", [1, 2 * FQ], bf16, kind="ExternalInput")
        bv = nc.dram_tensor("bv", [1, FQ], bf16, kind="ExternalInput")
        onesd = nc.dram_tensor("ones", [1, Cp], bf16, kind="ExternalInput")
    outT = nc.dram_tensor("outT", [D, Cq], bf16, kind="ExternalOutput")

    with tile.TileContext(nc) as tc:
        with (
            tc.tile_pool(name="big", bufs=1) as p_big,
            tc.tile_pool(name="att", bufs=2) as p_att,
            tc.tile_pool(name="pT", bufs=4) as p_pT,
            tc.tile_pool(name="outs", bufs=4) as p_out,
        ):
            qkT = p_big.tile([128, 8, Cp], bf16)
            vsb = p_big.tile([128, NC, HPC, VW], bf16)
            attnT = p_big.tile([128, HPC // 2, Cq], bf16)
            xTs = p_big.tile([128, KD, Cp], bf16)
            wqks = p_big.tile([128, KD, 2 * FQ], bf16)
            wvs = p_big.tile([128, KD, FQ], bf16)
            wps = p_big.tile([128, HPC // 2, D], bf16)
            for k in range(KD):
                nc.sync.dma_start(wqks[:, k], wqk[k * 128:(k + 1) * 128, :])
                nc.sync.dma_start(xTs[:, k], xT[k * 128:(k + 1) * 128, :])
            for k in range(KD):
                nc.sync.dma_start(wvs[:, k], wv[k * 128:(k + 1) * 128, :])
            keeps = p_big.tile([128, NC], f32)
            nc.sync.dma_start(keeps[:], keep[:])
            keepbs = p_big.tile([128, NC], bf16)
            nc.sync.dma_start(keepbs[:], keepb[:])
            nc.sync.dma_start(wps[:], wp[:].rearrange("(c p) n -> p c n", p=128))
            if with_bias:
                bqks = p_big.tile([1, 2 * FQ], bf16)
                nc.sync.dma_start(bqks[:], bqk[:])
                bvs = p_big.tile([1, FQ], bf16)
                nc.sync.dma_start(bvs[:], bv[:])
                ones = p_big.tile([1, Cp], bf16)
                nc.sync.dma_start(ones[:], onesd[:])

            def qk_group(pool, m, n0, nsz):
                # qkT[:, m, n0:n0+nsz] = (x @ Wqk[:, m-chunk] + b)^T
                ps = pool.tile([128, 512], f32, tag="psF")
                for k in range(KD):
                    nc.tensor.matmul(
                        ps[:, :nsz],
                        wqks[:, k, m * 128:(m + 1) * 128],
                        xTs[:, k, n0:n0 + nsz],
                        start=(k == 0), stop=(k == KD - 1) and not with_bias,
                    )
                if with_bias:
                    nc.tensor.matmul(
                        ps[:, :nsz],
                        bqks[0:1, m * 128:(m + 1) * 128],
                        ones[0:1, n0:n0 + nsz],
                        start=False, stop=True,
                    )
                nc.vector.tensor_copy(qkT[:, m, n0:n0 + nsz], ps[:, :nsz])

            def v_group(pool, c):
                # vsb[:, c, :, 64:128] = ((x @ Wv + bv) * keep)^chunk c
                ps = pool.tile([128, 512], f32, tag="psF")
                for k in range(KD):
                    nc.tensor.matmul(
                        ps[:],
                        xTs[:, k, c * 128:(c + 1) * 128],
                        wvs[:, k, :],
                        start=(k == 0), stop=(k == KD - 1) and not with_bias,
                    )
                if with_bias:
                    nc.tensor.matmul(
                        ps[:], ones[0:1, c * 128:(c + 1) * 128], bvs[0:1, :],
                        start=False, stop=True,
                    )
                nc.vector.tensor_scalar_mul(
                    vsb[:, c, :, 64:64 + HD], ps[:], keeps[:, c:c + 1]
                )

            def c_group(pool, m, n0, nsz):
                # outT[m-chunk, n0:n0+nsz] = (attn @ Wproj[g-part])^T
                ps = pool.tile([128, 512], f32, tag="psF")
                for j in range(HPC // 2):
                    nc.tensor.matmul(
                        ps[:, :nsz],
                        wps[:, j, m * 128:(m + 1) * 128],
                        attnT[:, j, n0:n0 + nsz],
                        start=(j == 0), stop=(j == HPC // 2 - 1),
                    )
                st = p_out.tile([128, 512], bf16, tag="st")
                nc.vector.tensor_copy(st[:, :nsz], ps[:, :nsz])
                nc.sync.dma_start(
                    outT[m * 128:(m + 1) * 128, n0:n0 + nsz], st[:, :nsz]
                )

            def norm_tail(bs):
                # softmax divide, entirely off the PE; denominator is on
                # PSUM partition 0 (the only partition the DVE
                # fast-reciprocal reads correctly from PSUM).
                avs, hp, q0, qsz = bs["avs"], bs["hp"], bs["q0"], bs["qsz"]
                bcss = []
                for hi in range(2):
                    rec = p_att.tile([1, 512], f32, tag=f"rec{hi}")
                    nc.vector.reciprocal_approx_fast(
                        rec[0:1, :qsz], avs[hi][0:1, :qsz])
                    recb = p_att.tile([1, 512], bf16, tag=f"recb{hi}")
                    nc.vector.tensor_copy(recb[0:1, :qsz], rec[0:1, :qsz])
                    bcs = p_att.tile([64, 512], bf16, tag=f"bcs{hi}")
                    nc.gpsimd.partition_broadcast(bcs[:, :qsz], recb[0:1, :qsz])
                    bcss.append(bcs)
                for hi in range(2):
                    nc.vector.tensor_mul(
                        attnT[hi * 64:hi * 64 + 64, hp, q0:q0 + qsz],
                        avs[hi][64:64 + HD, :qsz],
                        bcss[hi][:, :qsz],
                    )

            # ---- phase A: all projections ----
            with tc.tile_pool(name="psA", bufs=6, space="PSUM") as psA:
                # preload the ACT Exp table while the ACT is idle so the
                # first attention exp doesn't eat the 1.3us table load
                warm = p_att.tile([1, 1], f32, tag="warm", bufs=1)
                nc.scalar.activation(warm[:], keeps[0:1, 0:1], Exp)
                for m in range(4, 8):
                    for n0, nsz in kgroups:
                        qk_group(psA, m, n0, nsz)
                for m in range(4):
                    for n0, nsz in qgroups:
                        qk_group(psA, m, n0, nsz)
                for j in range(HPC):
                    nc.vector.tensor_copy(vsb[:, :, j, 0:1], keepbs[:])
                    nc.vector.memset(vsb[:, :, j, 1:64], 0)
                for c in range(NC):
                    v_group(psA, c)

            # ---- phase B: attention stream ----
            with (
                tc.tile_pool(name="psS", bufs=2, space="PSUM") as psS,
                tc.tile_pool(name="psAV", bufs=3, space="PSUM") as psAV,
            ):
                def S_exp(bs, kc):
                    hp, q0, qsz = bs["hp"], bs["q0"], bs["qsz"]
                    ss = psS.tile([128, 2, 512], f32, tag="ss")
                    for hi, h in enumerate(bs["heads"]):
                        lo = hi * 64
                        nc.tensor.matmul(
                            ss[:, hi, :qsz],
                            qkT[lo:lo + 64, 4 + hp, kc * 128:(kc + 1) * 128],
                            qkT[lo:lo + 64, hp, q0:q0 + qsz],
                            start=True, stop=True,
                        )
                    pT = p_pT.tile([128, 2, 512], bf16, tag="pT")
                    nc.scalar.activation(
                        pT[:, :, :qsz], ss[:, :, :qsz], Exp, scale=0.125
                    )
                    bs["pTs"][kc] = pT

                def AV(bs, kc):
                    qsz = bs["qsz"]
                    for hi, h in enumerate(bs["heads"]):
                        nc.tensor.matmul(
                            bs["avs"][hi][:, :qsz],
                            vsb[:, kc, h, :],
                            bs["pTs"][kc][:, hi, :qsz],
                            start=(kc == 0), stop=(kc == NC - 1),
                        )
                    bs["pTs"][kc] = None

                def new_block(gi, hp):
                    q0, qsz = qgroups[gi]
                    return {
                        "gi": gi, "hp": hp, "q0": q0, "qsz": qsz,
                        "heads": (2 * hp, 2 * hp + 1),
                        "avs": [psAV.tile([VW, 512], f32, tag="av",
                                          name=f"av_{gi}_{hp}_{hi}")
                                for hi in range(2)],
                        "pTs": [None] * NC,
                    }

                prev = None
                for gi in range(G):
                    for hp in range(4):
                        cur = new_block(gi, hp)
                        if prev is None:
                            S_exp(cur, 0)
                            S_exp(cur, 1)
                        else:
                            # boundary: the ACT still owes the last two
                            # exps; S'(0)/S'(1) interleave between the AV
                            # tail pairs to keep the exp pipeline fed.
                            AV(prev, NC - 2)
                            S_exp(cur, 0)
                            AV(prev, NC - 1)
                            S_exp(cur, 1)
                            norm_tail(prev)
                        for kc in range(2, NC):
                            S_exp(cur, kc)
                            AV(cur, kc - 2)
                        prev = cur
                AV(prev, NC - 2)
                AV(prev, NC - 1)
                norm_tail(prev)

            # ---- phase C: output projection ----
            with tc.tile_pool(name="psC", bufs=5, space="PSUM") as psC:
                for n0, nsz in qgroups[:-1]:
                    for m in range(8):
                        c_group(psC, m, n0, nsz)
                ln0, lnsz = qgroups[-1]
                for m in range(8):
                    c_group(psC, m, ln0, lnsz)

    nc.finalize()
    return nc


def _get_nc(Cp, Cq, with_bias):
    key = (Cp, Cq, with_bias)
    if key not in _NC_CACHE:
        _NC_CACHE[key] = _build_nc(Cp, Cq, with_bias)
    return _NC_CACHE[key]


def kernel(x, mask_ind, Wqkv, bqkv, Wproj, bproj, **_unused):
    from concourse.bass_utils import run_bass_kernel_spmd

    x = np.asarray(x, dtype=np.float32)
    mask_ind = np.asarray(mask_ind)
    Wqkv = np.asarray(Wqkv, dtype=np.float32)
    bqkv = np.asarray(bqkv, dtype=np.float32)
    Wproj = np.asarray(Wproj, dtype=np.float32)
    bproj = np.asarray(bproj, dtype=np.float32)

    # kept-token sets per batch (matches reference _keep_mask semantics)
    idx = []
    for b in range(B):
        mi = mask_ind[b]
        mi = mi[mi >= 0]
        mi = np.clip(mi, 0, C - 1)
        idx.append(np.unique(mi).astype(np.int64))
    nmax = max(len(u) for u in idx)
    Cp = max(384, ((nmax + 127) // 128) * 128)
    Cq = min(Cp, max(384, ((nmax + 31) // 32) * 32))
    NC = Cp // 128

    with_bias = bool(np.any(bqkv != 0.0))
    nc = _get_nc(Cp, Cq, with_bias)

    in_maps = []
    for core in range(N_CORES):
        b, g = core // 2, core % 2
        u = idx[b]
        n = len(u)
        xk = np.zeros((Cp, D), dtype=np.float32)
        xk[:n] = x[b, u]
        keep = np.zeros(Cp, dtype=np.float32)
        keep[:n] = 1.0
        qs, ks, vs = g * FQ, D + g * FQ, 2 * D + g * FQ
        wqk = np.concatenate(
            [Wqkv[:, qs:qs + FQ], Wqkv[:, ks:ks + FQ]], axis=1
        )
        im = {
            "xT": np.ascontiguousarray(xk.T).astype(BF16),
            "wqk": np.ascontiguousarray(wqk).astype(BF16),
            "wv": np.ascontiguousarray(Wqkv[:, vs:vs + FQ]).astype(BF16),
            "wp": np.ascontiguousarray(Wproj[g * FQ:(g + 1) * FQ, :]).astype(BF16),
            "keep": np.ascontiguousarray(keep.reshape(NC, 128).T),
            "keepb": np.ascontiguousarray(keep.reshape(NC, 128).T).astype(BF16),
        }
        if with_bias:
            bqk = np.concatenate([bqkv[qs:qs + FQ], bqkv[ks:ks + FQ]])
            im["bqk"] = bqk.reshape(1, -1).astype(BF16)
            im["bv"] = bqkv[vs:vs + FQ].reshape(1, -1).astype(BF16)
            im["ones"] = np.ones((1, Cp), dtype=BF16)
        in_maps.append(im)

    global _last_in_maps
    _last_in_maps = in_maps
    res = run_bass_kernel_spmd(nc, in_maps, core_ids=list(range(N_CORES)))

    out = np.broadcast_to(bproj, (B, C, D)).copy()
    for b in range(B):
        u = idx[b]
        n = len(u)
        comb = (res.results[2 * b]["outT"].astype(np.float32) +
                res.results[2 * b + 1]["outT"].astype(np.float32))
        out[b, u] += comb.T[:n]
    return out
